# revision 29
# baseline (speedup 1.0000x reference)
"""Paged GQA decode attention (vLLM-style) on 8 Trainium2 NeuronCores.

Problem (hardcoded shapes):
  query       (16, 32, 128) f32     16 seqs, 32 q heads, head 128
  key/value   (16, 8, 128)  f32     new decode token per seq, 8 kv heads
  key_cache   (4096, 16, 8, 128)    paged KV cache, block 16, 4096 blocks
  value_cache (4096, 16, 8, 128)
  block_tables(16, 256) i32         per-seq physical block list
  seq_lens    (16,) i32             context length incl. new token
  out         (16, 4096) f32        attention output, heads*head flattened

Sharding: tensor-parallel over the 8 kv heads -> core h owns kv head h and
its 4 query heads (GQA group = 4). Block tables / seq_lens replicated and
burned into the (identical-across-cores) instruction stream at build time.

Per-core algorithm (scoresT orientation, no max-subtraction -- scores are
~N(0,1) after the 1/sqrt(128) scale so exp never overflows):
  per seq s, per 128-token chunk t:
    scoresT[tok,4] = matmul(lhsT=K^T[head,tok] chunk, rhs=Q^T[head,4])
    probsT = exp(scale*scoresT + bias)      (ACT; bias column masks the tail)
    out[4,129]  += matmul(lhsT=probsT[tok,4], rhs=V[tok,129])   (PSUM accum)
  column 128 of V is a baked 1.0 -> out[:,128] is the softmax denominator.
  final: out[:, :128] * reciprocal(out[:, 128]).

Layouts prepared on the host (part of sharding):
  ktp  [128, 65536] f32  K^T: ktp[d, slot]  (slot = block*16 + offset)
  vp   [128, 512, 129]   V:  vp[p, C, d] = V[slot=128*C+p, d]; vp[p,C,128]=1
  qT   [128, 64]         qT[d, 4*s+g] = query[s, 4h+g, d]
  nkT  [128, 16]         new k transposed;  nv [16, 128] new v
  ebias[128, 32]         exp bias: col 2s = 0-vector, col 2s+1 = tail mask
K/V reach SBUF as bf16 via gpsimd cast-DMA (HBM traffic stays f32). The new
token's K/V is spliced into the SBUF tiles (device-side cache insert) before
the matmuls; the stale cache slot is the only masked-in-range position... it
is overwritten, and positions >= L get exp bias -30000 -> prob 0.
"""

import math

import numpy as np

NUM_SEQS = 16
NUM_HEADS = 32
NUM_KV = 8
HEAD = 128
BLOCK_SIZE = 16
NUM_BLOCKS = 4096
TOT_SLOTS = NUM_BLOCKS * BLOCK_SIZE  # 65536
GROUP = NUM_HEADS // NUM_KV  # 4
N_CORES = 8
CHUNK = 128  # tokens per matmul chunk
MAX_CHUNKS = 512  # TOT_SLOTS / CHUNK
SEQ_MAX_CHUNKS = 32  # 4096-token max context / 128

_BUILD_CACHE = {}

# Store the sharded KV cache in HBM as bf16. TensorE-facing tensors are bf16
# either way (without this flag the f32->bf16 cast happens inside the SWDGE
# DMA), so the SBUF values and the output are identical -- this only halves
# the HBM traffic.
KV_BF16 = True


def _slot_runs(block_tables, s, nchunks):
    """Physical-slot layout for tokens [0, nchunks*128) of seq s, coalesced
    into maximal runs of consecutive slots. Returns list of (dst_tok, slot0,
    length)."""
    nblk = nchunks * (CHUNK // BLOCK_SIZE)
    blocks = np.asarray(block_tables[s, :nblk], dtype=np.int64)
    slots = (blocks[:, None] * BLOCK_SIZE + np.arange(BLOCK_SIZE)[None, :]).reshape(-1)
    runs = []
    start = 0
    for i in range(1, len(slots) + 1):
        if i == len(slots) or slots[i] != slots[i - 1] + 1:
            runs.append((start, int(slots[start]), i - start))
            start = i
    return runs


def _build_bass(seq_lens, block_tables):
    import concourse.bacc as bacc
    import concourse.mybir as mybir
    import concourse.tile as tile

    f32 = mybir.dt.float32
    bf16 = mybir.dt.bfloat16
    Exp = mybir.ActivationFunctionType.Exp
    scale = 1.0 / math.sqrt(HEAD)

    seq_lens = [int(x) for x in seq_lens]
    nch = [int(math.ceil(L / CHUNK)) for L in seq_lens]

    kv_dt = bf16 if KV_BF16 else f32

    nc = bacc.Bacc()
    qT_d = nc.dram_tensor("qT", [HEAD, NUM_SEQS * GROUP], f32, kind="ExternalInput")
    ktp_d = nc.dram_tensor("ktp", [HEAD, TOT_SLOTS], kv_dt, kind="ExternalInput")
    vp_d = nc.dram_tensor("vp", [CHUNK, MAX_CHUNKS, HEAD + 1], kv_dt, kind="ExternalInput")
    nkT_d = nc.dram_tensor("nkT", [HEAD, NUM_SEQS], f32, kind="ExternalInput")
    nv_d = nc.dram_tensor("nv", [NUM_SEQS, HEAD], f32, kind="ExternalInput")
    eb_d = nc.dram_tensor("ebias", [CHUNK, 2 * NUM_SEQS], f32, kind="ExternalInput")
    out_d = nc.dram_tensor("out", [GROUP, NUM_SEQS, HEAD], f32, kind="ExternalOutput")

    with tile.TileContext(nc) as tc:
        with (
            tc.tile_pool(name="consts", bufs=1) as cpool,
            tc.tile_pool(name="kt", bufs=3) as kt_pool,
            tc.tile_pool(name="v", bufs=3) as v_pool,
            tc.tile_pool(name="probs", bufs=3) as p_pool,
            tc.tile_pool(name="fin", bufs=1) as fin_pool,
            tc.tile_pool(name="scps", bufs=3, space="PSUM") as sc_pool,
            tc.tile_pool(name="ops", bufs=3, space="PSUM") as o_pool,
        ):
            # const loads stay off gpsimd so SWDGE starts emitting the first
            # big K/V descriptors immediately; cast f32->bf16 on DVE instead
            qT_f = cpool.tile([HEAD, NUM_SEQS * GROUP], f32)
            nc.sync.dma_start(qT_f[:], qT_d[:])
            qT_sb = cpool.tile([HEAD, NUM_SEQS * GROUP], bf16)
            nc.vector.tensor_copy(qT_sb[:], qT_f[:])
            eb_sb = cpool.tile([CHUNK, 2 * NUM_SEQS], f32)
            nc.sync.dma_start(eb_sb[:], eb_d[:])
            nkT_f = cpool.tile([HEAD, NUM_SEQS], f32)
            nc.sync.dma_start(nkT_f[:], nkT_d[:])
            nkT_sb = cpool.tile([HEAD, NUM_SEQS], bf16)
            nc.vector.tensor_copy(nkT_sb[:], nkT_f[:])
            nv_f = cpool.tile([NUM_SEQS, HEAD], f32)
            nc.sync.dma_start(nv_f[:], nv_d[:])
            nv_sb = cpool.tile([NUM_SEQS, HEAD], bf16)
            nc.vector.tensor_copy(nv_sb[:], nv_f[:])
            stage = fin_pool.tile([GROUP, NUM_SEQS, HEAD + 1], f32)
            rd = fin_pool.tile([GROUP, NUM_SEQS], f32)
            osb = fin_pool.tile([GROUP, NUM_SEQS, HEAD], f32)

            # longest sequences first: the tail of the kernel is the last
            # seq's compute after its DMA lands -- make that the shortest
            order = sorted(range(NUM_SEQS), key=lambda s: -seq_lens[s])
            for si, s in enumerate(order):
                L = seq_lens[s]
                n = nch[s]
                last = L - 1

                kt = kt_pool.tile([HEAD, SEQ_MAX_CHUNKS * CHUNK], bf16, tag="kt")
                vt = v_pool.tile([CHUNK, SEQ_MAX_CHUNKS, HEAD + 1], bf16, tag="v")

                # both big streams ride the SP HWDGE ring, which runs no
                # compute-waiting instructions -- the ACT sequencer is left
                # to the exps alone. f32 mode: SWDGE cast-DMAs on gpsimd.
                if KV_BF16:
                    k_dma = v_dma = nc.sync.dma_start
                else:
                    k_dma = v_dma = nc.gpsimd.dma_start

                runs = _slot_runs(block_tables, s, n)
                for dst, slot0, ln in runs:
                    k_dma(kt[:, dst : dst + ln], ktp_d[:, slot0 : slot0 + ln])
                if len(runs) == 1 and runs[0][1] % CHUNK == 0:
                    c0 = runs[0][1] // CHUNK
                    v_dma(vt[:, :n, :], vp_d[:, c0 : c0 + n, :])
                else:
                    # general path: one DMA per 16-token block (block-aligned
                    # slots never straddle a 128-row physical chunk)
                    for dst, slot0, ln in runs:
                        for o in range(0, ln, BLOCK_SIZE):
                            sl = slot0 + o
                            dt_ = dst + o
                            v_dma(
                                vt[dt_ % CHUNK : dt_ % CHUNK + BLOCK_SIZE, dt_ // CHUNK, :],
                                vp_d[sl % CHUNK : sl % CHUNK + BLOCK_SIZE, sl // CHUNK, : HEAD + 1],
                            )

                # splice the new token's K/V over the stale cache slot
                # (tiny transfers stay off the HWDGE rings -- their completion
                # latency would head-of-line-block the big K/V streams)
                nc.vector.tensor_copy(kt[:, last : last + 1], nkT_sb[:, s : s + 1])
                r, c_last = last % CHUNK, last // CHUNK
                nc.gpsimd.dma_start(vt[r : r + 1, c_last, :HEAD], nv_sb[s : s + 1, :])

                sc = sc_pool.tile([CHUNK, SEQ_MAX_CHUNKS * GROUP], f32, tag="sc")
                for t in range(n):
                    nc.tensor.matmul(
                        sc[:, GROUP * t : GROUP * (t + 1)],
                        kt[:, CHUNK * t : CHUNK * (t + 1)],
                        qT_sb[:, GROUP * s : GROUP * (s + 1)],
                        start=True,
                        stop=True,
                    )

                probs = p_pool.tile([CHUNK, SEQ_MAX_CHUNKS * GROUP], bf16, tag="probs")
                if n > 1:
                    nc.scalar.activation(
                        probs[:, : GROUP * (n - 1)],
                        sc[:, : GROUP * (n - 1)],
                        Exp,
                        bias=eb_sb[:, 2 * s : 2 * s + 1],
                        scale=scale,
                    )
                nc.scalar.activation(
                    probs[:, GROUP * (n - 1) : GROUP * n],
                    sc[:, GROUP * (n - 1) : GROUP * n],
                    Exp,
                    bias=eb_sb[:, 2 * s + 1 : 2 * s + 2],
                    scale=scale,
                )

                acc = o_pool.tile([GROUP, HEAD + 1], f32, tag="acc")
                for t in range(n):
                    nc.tensor.matmul(
                        acc[:],
                        probs[:, GROUP * t : GROUP * (t + 1)],
                        vt[:, t, :],
                        start=(t == 0),
                        stop=(t == n - 1),
                    )
                # per-seq finalize so the output DMA overlaps later seqs
                nc.vector.tensor_copy(stage[:, s, :], acc[:])
                nc.vector.reciprocal(rd[:, s : s + 1], stage[:, s, HEAD:])
                nc.vector.tensor_tensor(
                    osb[:, s, :],
                    stage[:, s, :HEAD],
                    rd[:, s : s + 1].to_broadcast((GROUP, HEAD)),
                    mybir.AluOpType.mult,
                )
                nc.gpsimd.dma_start(out_d[:, s, :], osb[:, s, :])

    nc.finalize()
    return nc


def _prep_inputs(query, key, value, key_cache, value_cache, seq_lens):
    """Per-core host shards. Returns list of 8 dicts of f32 arrays."""
    query = np.asarray(query, dtype=np.float32)
    key = np.asarray(key, dtype=np.float32)
    value = np.asarray(value, dtype=np.float32)
    key_cache = np.asarray(key_cache, dtype=np.float32)
    value_cache = np.asarray(value_cache, dtype=np.float32)
    seq_lens = np.asarray(seq_lens)

    # exp bias: for each seq a zero column (full chunks) and a tail-mask
    # column for the final chunk (rows >= L - 128*(nch-1) get -30000)
    eb = np.zeros((CHUNK, 2 * NUM_SEQS), dtype=np.float32)
    for s in range(NUM_SEQS):
        L = int(seq_lens[s])
        n = int(math.ceil(L / CHUNK))
        v = L - CHUNK * (n - 1)
        eb[v:, 2 * s + 1] = -30000.0

    kc = key_cache.reshape(TOT_SLOTS, NUM_KV, HEAD)
    vc = value_cache.reshape(TOT_SLOTS, NUM_KV, HEAD)
    if KV_BF16:
        import ml_dtypes

        kv_np = ml_dtypes.bfloat16
    else:
        kv_np = np.float32

    in_maps = []
    for h in range(N_CORES):
        ktp = np.ascontiguousarray(kc[:, h, :].T.astype(kv_np))  # [128, 65536]
        vp = np.empty((CHUNK, MAX_CHUNKS, HEAD + 1), dtype=kv_np)
        vp[:, :, :HEAD] = (
            vc[:, h, :].reshape(MAX_CHUNKS, CHUNK, HEAD).transpose(1, 0, 2).astype(kv_np)
        )
        vp[:, :, HEAD] = 1.0
        qT = np.ascontiguousarray(
            query[:, GROUP * h : GROUP * (h + 1), :].reshape(NUM_SEQS * GROUP, HEAD).T
        )
        nkT = np.ascontiguousarray(key[:, h, :].T)  # [128, 16]
        nv = np.ascontiguousarray(value[:, h, :])  # [16, 128]
        in_maps.append(
            {"qT": qT, "ktp": ktp, "vp": vp, "nkT": nkT, "nv": nv, "ebias": eb}
        )
    return in_maps


def kernel(query, key, value, key_cache, value_cache, block_tables, seq_lens):
    from concourse.bass_utils import run_bass_kernel_spmd

    block_tables = np.asarray(block_tables)
    seq_lens_np = np.asarray(seq_lens)

    cache_key = (tuple(int(x) for x in seq_lens_np), block_tables.tobytes())
    nc = _BUILD_CACHE.get(cache_key)
    if nc is None:
        nc = _build_bass(seq_lens_np, block_tables)
        _BUILD_CACHE[cache_key] = nc

    in_maps = _prep_inputs(query, key, value, key_cache, value_cache, seq_lens_np)
    res = run_bass_kernel_spmd(nc, in_maps, core_ids=list(range(N_CORES)))

    full = np.empty((NUM_SEQS, NUM_HEADS, HEAD), dtype=np.float32)
    for h in range(N_CORES):
        o = np.asarray(res.results[h]["out"])  # [GROUP, NUM_SEQS, HEAD]
        full[:, GROUP * h : GROUP * (h + 1), :] = o.transpose(1, 0, 2)
    return full.reshape(NUM_SEQS, NUM_HEADS * HEAD)


# revision 30
# speedup vs baseline: 1.3723x; 1.3723x over previous
"""Paged GQA decode attention (vLLM-style) on 8 Trainium2 NeuronCores.

Problem (hardcoded shapes):
  query       (16, 32, 128) f32     16 seqs, 32 q heads, head 128
  key/value   (16, 8, 128)  f32     new decode token per seq, 8 kv heads
  key_cache   (4096, 16, 8, 128)    paged KV cache, block 16, 4096 blocks
  value_cache (4096, 16, 8, 128)
  block_tables(16, 256) i32         per-seq physical block list
  seq_lens    (16,) i32             context length incl. new token
  out         (16, 4096) f32        attention output, heads*head flattened

Sharding: tensor-parallel over the 8 kv heads -> core h owns kv head h and
its 4 query heads (GQA group = 4). Block tables / seq_lens replicated and
burned into the (identical-across-cores) instruction stream at build time.

Per-core algorithm (scoresT orientation, no max-subtraction -- scores are
~N(0,1) after the 1/sqrt(128) scale so exp never overflows):
  per seq s, per 128-token chunk t:
    scoresT[tok,4] = matmul(lhsT=K^T[head,tok] chunk, rhs=Q^T[head,4])
    probsT = exp(scale*scoresT + bias)      (ACT; bias column masks the tail)
    out[4,129]  += matmul(lhsT=probsT[tok,4], rhs=V[tok,129])   (PSUM accum)
  column 128 of V is a baked 1.0 -> out[:,128] is the softmax denominator.
  final: out[:, :128] * reciprocal(out[:, 128]).

Layouts prepared on the host (part of sharding):
  ktp  [128, 65536] f32  K^T: ktp[d, slot]  (slot = block*16 + offset)
  vp   [128, 512, 129]   V:  vp[p, C, d] = V[slot=128*C+p, d]; vp[p,C,128]=1
  qT   [128, 64]         qT[d, 4*s+g] = query[s, 4h+g, d]
  nkT  [128, 16]         new k transposed;  nv [16, 128] new v
  ebias[128, 32]         exp bias: col 2s = 0-vector, col 2s+1 = tail mask
K/V reach SBUF as bf16 via gpsimd cast-DMA (HBM traffic stays f32). The new
token's K/V is spliced into the SBUF tiles (device-side cache insert) before
the matmuls; the stale cache slot is the only masked-in-range position... it
is overwritten, and positions >= L get exp bias -30000 -> prob 0.
"""

import math

import numpy as np

NUM_SEQS = 16
NUM_HEADS = 32
NUM_KV = 8
HEAD = 128
BLOCK_SIZE = 16
NUM_BLOCKS = 4096
TOT_SLOTS = NUM_BLOCKS * BLOCK_SIZE  # 65536
GROUP = NUM_HEADS // NUM_KV  # 4
N_CORES = 8
CHUNK = 128  # tokens per matmul chunk
MAX_CHUNKS = 512  # TOT_SLOTS / CHUNK
SEQ_MAX_CHUNKS = 32  # 4096-token max context / 128

_BUILD_CACHE = {}

# Store the sharded KV cache in HBM as bf16. TensorE-facing tensors are bf16
# either way (without this flag the f32->bf16 cast happens inside the SWDGE
# DMA), so the SBUF values and the output are identical -- this only halves
# the HBM traffic.
KV_BF16 = True


def _slot_runs(block_tables, s, nchunks):
    """Physical-slot layout for tokens [0, nchunks*128) of seq s, coalesced
    into maximal runs of consecutive slots. Returns list of (dst_tok, slot0,
    length)."""
    nblk = nchunks * (CHUNK // BLOCK_SIZE)
    blocks = np.asarray(block_tables[s, :nblk], dtype=np.int64)
    slots = (blocks[:, None] * BLOCK_SIZE + np.arange(BLOCK_SIZE)[None, :]).reshape(-1)
    runs = []
    start = 0
    for i in range(1, len(slots) + 1):
        if i == len(slots) or slots[i] != slots[i - 1] + 1:
            runs.append((start, int(slots[start]), i - start))
            start = i
    return runs


def _build_bass(seq_lens, block_tables):
    import concourse.bacc as bacc
    import concourse.mybir as mybir
    import concourse.tile as tile

    f32 = mybir.dt.float32
    bf16 = mybir.dt.bfloat16
    Exp = mybir.ActivationFunctionType.Exp
    scale = 1.0 / math.sqrt(HEAD)

    seq_lens = [int(x) for x in seq_lens]
    nch = [int(math.ceil(L / CHUNK)) for L in seq_lens]

    kv_dt = bf16 if KV_BF16 else f32

    nc = bacc.Bacc()
    qT_d = nc.dram_tensor("qT", [HEAD, NUM_SEQS * GROUP], f32, kind="ExternalInput")
    ktp_d = nc.dram_tensor("ktp", [HEAD, TOT_SLOTS], kv_dt, kind="ExternalInput")
    vp_d = nc.dram_tensor("vp", [CHUNK, MAX_CHUNKS, HEAD + 1], kv_dt, kind="ExternalInput")
    nkT_d = nc.dram_tensor("nkT", [HEAD, NUM_SEQS], f32, kind="ExternalInput")
    nv_d = nc.dram_tensor("nv", [NUM_SEQS, HEAD], f32, kind="ExternalInput")
    eb_d = nc.dram_tensor("ebias", [CHUNK, 2 * NUM_SEQS], f32, kind="ExternalInput")
    out_d = nc.dram_tensor("out", [GROUP, NUM_SEQS, HEAD], f32, kind="ExternalOutput")

    with tile.TileContext(nc) as tc:
        with (
            tc.tile_pool(name="consts", bufs=1) as cpool,
            tc.tile_pool(name="kt", bufs=3) as kt_pool,
            tc.tile_pool(name="v", bufs=3) as v_pool,
            tc.tile_pool(name="probs", bufs=3) as p_pool,
            tc.tile_pool(name="fin", bufs=1) as fin_pool,
            tc.tile_pool(name="scps", bufs=3, space="PSUM") as sc_pool,
            tc.tile_pool(name="ops", bufs=3, space="PSUM") as o_pool,
        ):
            # const loads stay off gpsimd so SWDGE starts emitting the first
            # big K/V descriptors immediately; cast f32->bf16 on DVE instead
            qT_f = cpool.tile([HEAD, NUM_SEQS * GROUP], f32)
            nc.sync.dma_start(qT_f[:], qT_d[:])
            qT_sb = cpool.tile([HEAD, NUM_SEQS * GROUP], bf16)
            nc.vector.tensor_copy(qT_sb[:], qT_f[:])
            eb_sb = cpool.tile([CHUNK, 2 * NUM_SEQS], f32)
            nc.sync.dma_start(eb_sb[:], eb_d[:])
            nkT_f = cpool.tile([HEAD, NUM_SEQS], f32)
            nc.sync.dma_start(nkT_f[:], nkT_d[:])
            nkT_sb = cpool.tile([HEAD, NUM_SEQS], bf16)
            nc.vector.tensor_copy(nkT_sb[:], nkT_f[:])
            nv_f = cpool.tile([NUM_SEQS, HEAD], f32)
            nc.sync.dma_start(nv_f[:], nv_d[:])
            nv_sb = cpool.tile([NUM_SEQS, HEAD], bf16)
            nc.vector.tensor_copy(nv_sb[:], nv_f[:])
            stage = fin_pool.tile([GROUP, NUM_SEQS, HEAD + 1], f32)
            rd = fin_pool.tile([GROUP, NUM_SEQS], f32)
            osb = fin_pool.tile([GROUP, NUM_SEQS, HEAD], f32)

            # longest sequences first: the tail of the kernel is the last
            # seq's compute after its DMA lands -- make that the shortest
            order = sorted(range(NUM_SEQS), key=lambda s: -seq_lens[s])
            for si, s in enumerate(order):
                L = seq_lens[s]
                n = nch[s]
                last = L - 1

                kt = kt_pool.tile([HEAD, SEQ_MAX_CHUNKS * CHUNK], bf16, tag="kt")
                vt = v_pool.tile([CHUNK, SEQ_MAX_CHUNKS, HEAD + 1], bf16, tag="v")

                # two parallel HWDGE rings: K on SP, V on ACT. (Both on one
                # ring serializes the stream: measured 147us vs 99us; V on
                # SWDGE couples with the gpsimd tiny-DMA stream: 129us.)
                if KV_BF16:
                    k_dma, v_dma = nc.sync.dma_start, nc.scalar.dma_start
                else:
                    k_dma = v_dma = nc.gpsimd.dma_start

                runs = _slot_runs(block_tables, s, n)
                for dst, slot0, ln in runs:
                    k_dma(kt[:, dst : dst + ln], ktp_d[:, slot0 : slot0 + ln])
                if len(runs) == 1 and runs[0][1] % CHUNK == 0:
                    c0 = runs[0][1] // CHUNK
                    v_dma(vt[:, :n, :], vp_d[:, c0 : c0 + n, :])
                else:
                    # general path: one DMA per 16-token block (block-aligned
                    # slots never straddle a 128-row physical chunk)
                    for dst, slot0, ln in runs:
                        for o in range(0, ln, BLOCK_SIZE):
                            sl = slot0 + o
                            dt_ = dst + o
                            v_dma(
                                vt[dt_ % CHUNK : dt_ % CHUNK + BLOCK_SIZE, dt_ // CHUNK, :],
                                vp_d[sl % CHUNK : sl % CHUNK + BLOCK_SIZE, sl // CHUNK, : HEAD + 1],
                            )

                # splice the new token's K/V over the stale cache slot
                # (tiny transfers stay off the HWDGE rings -- their completion
                # latency would head-of-line-block the big K/V streams)
                nc.vector.tensor_copy(kt[:, last : last + 1], nkT_sb[:, s : s + 1])
                r, c_last = last % CHUNK, last // CHUNK
                nc.gpsimd.dma_start(vt[r : r + 1, c_last, :HEAD], nv_sb[s : s + 1, :])

                sc = sc_pool.tile([CHUNK, SEQ_MAX_CHUNKS * GROUP], f32, tag="sc")
                for t in range(n):
                    nc.tensor.matmul(
                        sc[:, GROUP * t : GROUP * (t + 1)],
                        kt[:, CHUNK * t : CHUNK * (t + 1)],
                        qT_sb[:, GROUP * s : GROUP * (s + 1)],
                        start=True,
                        stop=True,
                    )

                probs = p_pool.tile([CHUNK, SEQ_MAX_CHUNKS * GROUP], bf16, tag="probs")
                if n > 1:
                    nc.scalar.activation(
                        probs[:, : GROUP * (n - 1)],
                        sc[:, : GROUP * (n - 1)],
                        Exp,
                        bias=eb_sb[:, 2 * s : 2 * s + 1],
                        scale=scale,
                    )
                nc.scalar.activation(
                    probs[:, GROUP * (n - 1) : GROUP * n],
                    sc[:, GROUP * (n - 1) : GROUP * n],
                    Exp,
                    bias=eb_sb[:, 2 * s + 1 : 2 * s + 2],
                    scale=scale,
                )

                acc = o_pool.tile([GROUP, HEAD + 1], f32, tag="acc")
                for t in range(n):
                    nc.tensor.matmul(
                        acc[:],
                        probs[:, GROUP * t : GROUP * (t + 1)],
                        vt[:, t, :],
                        start=(t == 0),
                        stop=(t == n - 1),
                    )
                # per-seq finalize so the output DMA overlaps later seqs
                nc.vector.tensor_copy(stage[:, s, :], acc[:])
                nc.vector.reciprocal(rd[:, s : s + 1], stage[:, s, HEAD:])
                nc.vector.tensor_tensor(
                    osb[:, s, :],
                    stage[:, s, :HEAD],
                    rd[:, s : s + 1].to_broadcast((GROUP, HEAD)),
                    mybir.AluOpType.mult,
                )
                nc.gpsimd.dma_start(out_d[:, s, :], osb[:, s, :])

    nc.finalize()
    return nc


def _prep_inputs(query, key, value, key_cache, value_cache, seq_lens):
    """Per-core host shards. Returns list of 8 dicts of f32 arrays."""
    query = np.asarray(query, dtype=np.float32)
    key = np.asarray(key, dtype=np.float32)
    value = np.asarray(value, dtype=np.float32)
    key_cache = np.asarray(key_cache, dtype=np.float32)
    value_cache = np.asarray(value_cache, dtype=np.float32)
    seq_lens = np.asarray(seq_lens)

    # exp bias: for each seq a zero column (full chunks) and a tail-mask
    # column for the final chunk (rows >= L - 128*(nch-1) get -30000)
    eb = np.zeros((CHUNK, 2 * NUM_SEQS), dtype=np.float32)
    for s in range(NUM_SEQS):
        L = int(seq_lens[s])
        n = int(math.ceil(L / CHUNK))
        v = L - CHUNK * (n - 1)
        eb[v:, 2 * s + 1] = -30000.0

    kc = key_cache.reshape(TOT_SLOTS, NUM_KV, HEAD)
    vc = value_cache.reshape(TOT_SLOTS, NUM_KV, HEAD)
    if KV_BF16:
        import ml_dtypes

        kv_np = ml_dtypes.bfloat16
    else:
        kv_np = np.float32

    in_maps = []
    for h in range(N_CORES):
        ktp = np.ascontiguousarray(kc[:, h, :].T.astype(kv_np))  # [128, 65536]
        vp = np.empty((CHUNK, MAX_CHUNKS, HEAD + 1), dtype=kv_np)
        vp[:, :, :HEAD] = (
            vc[:, h, :].reshape(MAX_CHUNKS, CHUNK, HEAD).transpose(1, 0, 2).astype(kv_np)
        )
        vp[:, :, HEAD] = 1.0
        qT = np.ascontiguousarray(
            query[:, GROUP * h : GROUP * (h + 1), :].reshape(NUM_SEQS * GROUP, HEAD).T
        )
        nkT = np.ascontiguousarray(key[:, h, :].T)  # [128, 16]
        nv = np.ascontiguousarray(value[:, h, :])  # [16, 128]
        in_maps.append(
            {"qT": qT, "ktp": ktp, "vp": vp, "nkT": nkT, "nv": nv, "ebias": eb}
        )
    return in_maps


def kernel(query, key, value, key_cache, value_cache, block_tables, seq_lens):
    from concourse.bass_utils import run_bass_kernel_spmd

    block_tables = np.asarray(block_tables)
    seq_lens_np = np.asarray(seq_lens)

    cache_key = (tuple(int(x) for x in seq_lens_np), block_tables.tobytes())
    nc = _BUILD_CACHE.get(cache_key)
    if nc is None:
        nc = _build_bass(seq_lens_np, block_tables)
        _BUILD_CACHE[cache_key] = nc

    in_maps = _prep_inputs(query, key, value, key_cache, value_cache, seq_lens_np)
    res = run_bass_kernel_spmd(nc, in_maps, core_ids=list(range(N_CORES)))

    full = np.empty((NUM_SEQS, NUM_HEADS, HEAD), dtype=np.float32)
    for h in range(N_CORES):
        o = np.asarray(res.results[h]["out"])  # [GROUP, NUM_SEQS, HEAD]
        full[:, GROUP * h : GROUP * (h + 1), :] = o.transpose(1, 0, 2)
    return full.reshape(NUM_SEQS, NUM_HEADS * HEAD)


# revision 31
# speedup vs baseline: 1.4920x; 1.0872x over previous
"""Paged GQA decode attention (vLLM-style) on 8 Trainium2 NeuronCores.

Problem (hardcoded shapes):
  query       (16, 32, 128) f32     16 seqs, 32 q heads, head 128
  key/value   (16, 8, 128)  f32     new decode token per seq, 8 kv heads
  key_cache   (4096, 16, 8, 128)    paged KV cache, block 16, 4096 blocks
  value_cache (4096, 16, 8, 128)
  block_tables(16, 256) i32         per-seq physical block list
  seq_lens    (16,) i32             context length incl. new token
  out         (16, 4096) f32        attention output, heads*head flattened

Sharding: tensor-parallel over the 8 kv heads -> core h owns kv head h and
its 4 query heads (GQA group = 4). Block tables / seq_lens replicated and
burned into the (identical-across-cores) instruction stream at build time.

Per-core algorithm (scoresT orientation, no max-subtraction -- scores are
~N(0,1) after the 1/sqrt(128) scale so exp never overflows):
  per seq s, per 128-token chunk t:
    scoresT[tok,4] = matmul(lhsT=K^T[head,tok] chunk, rhs=Q^T[head,4])
    probsT = exp(scale*scoresT + bias)      (ACT; bias column masks the tail)
    out[4,129]  += matmul(lhsT=probsT[tok,4], rhs=V[tok,129])   (PSUM accum)
  column 128 of V is a baked 1.0 -> out[:,128] is the softmax denominator.
  final: out[:, :128] * reciprocal(out[:, 128]).

Layouts prepared on the host (part of sharding):
  ktp  [128, 65536] f32  K^T: ktp[d, slot]  (slot = block*16 + offset)
  vp   [128, 512, 129]   V:  vp[p, C, d] = V[slot=128*C+p, d]; vp[p,C,128]=1
  qT   [128, 64]         qT[d, 4*s+g] = query[s, 4h+g, d]
  nkT  [128, 16]         new k transposed;  nv [16, 128] new v
  ebias[128, 32]         exp bias: col 2s = 0-vector, col 2s+1 = tail mask
TensorE consumes bf16 (FP32 matmul is 4x slower); with KV_BF16 the sharded
cache is stored bf16 in HBM (same SBUF values as the cast-during-DMA path,
half the traffic), K rides the SP HWDGE ring and V the ACT ring. The new
token's K/V is spliced into the SBUF tiles (device-side cache insert) before
the matmuls; the stale cache slot is overwritten, and positions >= L get exp
bias -30000 -> prob 0.
"""

import math

import numpy as np

NUM_SEQS = 16
NUM_HEADS = 32
NUM_KV = 8
HEAD = 128
BLOCK_SIZE = 16
NUM_BLOCKS = 4096
TOT_SLOTS = NUM_BLOCKS * BLOCK_SIZE  # 65536
GROUP = NUM_HEADS // NUM_KV  # 4
N_CORES = 8
CHUNK = 128  # tokens per matmul chunk
MAX_CHUNKS = 512  # TOT_SLOTS / CHUNK
SEQ_MAX_CHUNKS = 32  # 4096-token max context / 128

_BUILD_CACHE = {}

# Store the sharded KV cache in HBM as bf16. TensorE-facing tensors are bf16
# either way (without this flag the f32->bf16 cast happens inside the SWDGE
# DMA), so the SBUF values and the output are identical -- this only halves
# the HBM traffic.
KV_BF16 = True


def _slot_runs(block_tables, s, nchunks):
    """Physical-slot layout for tokens [0, nchunks*128) of seq s, coalesced
    into maximal runs of consecutive slots. Returns list of (dst_tok, slot0,
    length)."""
    nblk = nchunks * (CHUNK // BLOCK_SIZE)
    blocks = np.asarray(block_tables[s, :nblk], dtype=np.int64)
    slots = (blocks[:, None] * BLOCK_SIZE + np.arange(BLOCK_SIZE)[None, :]).reshape(-1)
    runs = []
    start = 0
    for i in range(1, len(slots) + 1):
        if i == len(slots) or slots[i] != slots[i - 1] + 1:
            runs.append((start, int(slots[start]), i - start))
            start = i
    return runs


def _build_bass(seq_lens, block_tables):
    import concourse.bacc as bacc
    import concourse.mybir as mybir
    import concourse.tile as tile

    f32 = mybir.dt.float32
    bf16 = mybir.dt.bfloat16
    Exp = mybir.ActivationFunctionType.Exp
    scale = 1.0 / math.sqrt(HEAD)

    seq_lens = [int(x) for x in seq_lens]
    nch = [int(math.ceil(L / CHUNK)) for L in seq_lens]

    kv_dt = bf16 if KV_BF16 else f32

    nc = bacc.Bacc()
    qT_d = nc.dram_tensor("qT", [HEAD, NUM_SEQS * GROUP], f32, kind="ExternalInput")
    ktp_d = nc.dram_tensor("ktp", [HEAD, TOT_SLOTS], kv_dt, kind="ExternalInput")
    vp_d = nc.dram_tensor("vp", [CHUNK, MAX_CHUNKS, HEAD + 1], kv_dt, kind="ExternalInput")
    nkT_d = nc.dram_tensor("nkT", [HEAD, NUM_SEQS], f32, kind="ExternalInput")
    nv_d = nc.dram_tensor("nv", [NUM_SEQS, HEAD], f32, kind="ExternalInput")
    eb_d = nc.dram_tensor("ebias", [CHUNK, 2 * NUM_SEQS], f32, kind="ExternalInput")
    out_d = nc.dram_tensor("out", [GROUP, NUM_SEQS, HEAD], f32, kind="ExternalOutput")

    with tile.TileContext(nc) as tc:
        with (
            tc.tile_pool(name="consts", bufs=1) as cpool,
            tc.tile_pool(name="kt", bufs=3) as kt_pool,
            tc.tile_pool(name="v", bufs=3) as v_pool,
            tc.tile_pool(name="probs", bufs=3) as p_pool,
            tc.tile_pool(name="fin", bufs=1) as fin_pool,
            tc.tile_pool(name="scps", bufs=3, space="PSUM") as sc_pool,
            tc.tile_pool(name="ops", bufs=3, space="PSUM") as o_pool,
        ):
            # const loads stay off gpsimd so SWDGE starts emitting the first
            # big K/V descriptors immediately; cast f32->bf16 on DVE instead
            qT_f = cpool.tile([HEAD, NUM_SEQS * GROUP], f32)
            nc.sync.dma_start(qT_f[:], qT_d[:])
            qT_sb = cpool.tile([HEAD, NUM_SEQS * GROUP], bf16)
            nc.vector.tensor_copy(qT_sb[:], qT_f[:])
            eb_sb = cpool.tile([CHUNK, 2 * NUM_SEQS], f32)
            nc.sync.dma_start(eb_sb[:], eb_d[:])
            nkT_f = cpool.tile([HEAD, NUM_SEQS], f32)
            nc.sync.dma_start(nkT_f[:], nkT_d[:])
            nkT_sb = cpool.tile([HEAD, NUM_SEQS], bf16)
            nc.vector.tensor_copy(nkT_sb[:], nkT_f[:])
            nv_f = cpool.tile([NUM_SEQS, HEAD], f32)
            nc.sync.dma_start(nv_f[:], nv_d[:])
            nv_sb = cpool.tile([NUM_SEQS, HEAD], bf16)
            nc.vector.tensor_copy(nv_sb[:], nv_f[:])
            stage = fin_pool.tile([GROUP, NUM_SEQS, HEAD + 1], f32)
            rd = fin_pool.tile([GROUP, NUM_SEQS], f32)
            osb = fin_pool.tile([GROUP, NUM_SEQS, HEAD], f32)

            # longest sequences first: the tail of the kernel is the last
            # seq's compute after its DMA lands -- make that the shortest
            order = sorted(range(NUM_SEQS), key=lambda s: -seq_lens[s])
            for si, s in enumerate(order):
                L = seq_lens[s]
                n = nch[s]
                last = L - 1

                kt = kt_pool.tile([HEAD, SEQ_MAX_CHUNKS * CHUNK], bf16, tag="kt")
                vt = v_pool.tile([CHUNK, SEQ_MAX_CHUNKS, HEAD + 1], bf16, tag="v")

                # two parallel HWDGE rings: K on SP, V on ACT. (Both on one
                # ring serializes the stream: measured 147us vs 99us; V on
                # SWDGE couples with the gpsimd tiny-DMA stream: 129us.)
                if KV_BF16:
                    k_dma, v_dma = nc.sync.dma_start, nc.scalar.dma_start
                else:
                    k_dma = v_dma = nc.gpsimd.dma_start

                runs = _slot_runs(block_tables, s, n)
                for dst, slot0, ln in runs:
                    k_dma(kt[:, dst : dst + ln], ktp_d[:, slot0 : slot0 + ln])
                if len(runs) == 1 and runs[0][1] % CHUNK == 0:
                    c0 = runs[0][1] // CHUNK
                    v_dma(vt[:, :n, :], vp_d[:, c0 : c0 + n, :])
                else:
                    # general path: one DMA per 16-token block (block-aligned
                    # slots never straddle a 128-row physical chunk)
                    for dst, slot0, ln in runs:
                        for o in range(0, ln, BLOCK_SIZE):
                            sl = slot0 + o
                            dt_ = dst + o
                            v_dma(
                                vt[dt_ % CHUNK : dt_ % CHUNK + BLOCK_SIZE, dt_ // CHUNK, :],
                                vp_d[sl % CHUNK : sl % CHUNK + BLOCK_SIZE, sl // CHUNK, : HEAD + 1],
                            )

                # splice the new token's K/V over the stale cache slot
                # (tiny transfers stay off the HWDGE rings -- their completion
                # latency would head-of-line-block the big K/V streams)
                nc.vector.tensor_copy(kt[:, last : last + 1], nkT_sb[:, s : s + 1])
                r, c_last = last % CHUNK, last // CHUNK
                nc.gpsimd.dma_start(vt[r : r + 1, c_last, :HEAD], nv_sb[s : s + 1, :])

                sc = sc_pool.tile([CHUNK, SEQ_MAX_CHUNKS * GROUP], f32, tag="sc")
                for t in range(n):
                    nc.tensor.matmul(
                        sc[:, GROUP * t : GROUP * (t + 1)],
                        kt[:, CHUNK * t : CHUNK * (t + 1)],
                        qT_sb[:, GROUP * s : GROUP * (s + 1)],
                        start=True,
                        stop=True,
                    )

                probs = p_pool.tile([CHUNK, SEQ_MAX_CHUNKS * GROUP], bf16, tag="probs")
                if n > 1:
                    nc.scalar.activation(
                        probs[:, : GROUP * (n - 1)],
                        sc[:, : GROUP * (n - 1)],
                        Exp,
                        bias=eb_sb[:, 2 * s : 2 * s + 1],
                        scale=scale,
                    )
                nc.scalar.activation(
                    probs[:, GROUP * (n - 1) : GROUP * n],
                    sc[:, GROUP * (n - 1) : GROUP * n],
                    Exp,
                    bias=eb_sb[:, 2 * s + 1 : 2 * s + 2],
                    scale=scale,
                )

                acc = o_pool.tile([GROUP, HEAD + 1], f32, tag="acc")
                for t in range(n):
                    nc.tensor.matmul(
                        acc[:],
                        probs[:, GROUP * t : GROUP * (t + 1)],
                        vt[:, t, :],
                        start=(t == 0),
                        stop=(t == n - 1),
                    )
                # per-seq finalize so the output DMA overlaps later seqs
                nc.vector.tensor_copy(stage[:, s, :], acc[:])
                nc.vector.reciprocal(rd[:, s : s + 1], stage[:, s, HEAD:])
                nc.vector.tensor_tensor(
                    osb[:, s, :],
                    stage[:, s, :HEAD],
                    rd[:, s : s + 1].to_broadcast((GROUP, HEAD)),
                    mybir.AluOpType.mult,
                )
                nc.gpsimd.dma_start(out_d[:, s, :], osb[:, s, :])

    nc.finalize()
    return nc


def _prep_inputs(query, key, value, key_cache, value_cache, seq_lens):
    """Per-core host shards. Returns list of 8 dicts of f32 arrays."""
    query = np.asarray(query, dtype=np.float32)
    key = np.asarray(key, dtype=np.float32)
    value = np.asarray(value, dtype=np.float32)
    key_cache = np.asarray(key_cache, dtype=np.float32)
    value_cache = np.asarray(value_cache, dtype=np.float32)
    seq_lens = np.asarray(seq_lens)

    # exp bias: for each seq a zero column (full chunks) and a tail-mask
    # column for the final chunk (rows >= L - 128*(nch-1) get -30000)
    eb = np.zeros((CHUNK, 2 * NUM_SEQS), dtype=np.float32)
    for s in range(NUM_SEQS):
        L = int(seq_lens[s])
        n = int(math.ceil(L / CHUNK))
        v = L - CHUNK * (n - 1)
        eb[v:, 2 * s + 1] = -30000.0

    kc = key_cache.reshape(TOT_SLOTS, NUM_KV, HEAD)
    vc = value_cache.reshape(TOT_SLOTS, NUM_KV, HEAD)
    if KV_BF16:
        import ml_dtypes

        kv_np = ml_dtypes.bfloat16
    else:
        kv_np = np.float32

    in_maps = []
    for h in range(N_CORES):
        ktp = np.ascontiguousarray(kc[:, h, :].T.astype(kv_np))  # [128, 65536]
        vp = np.empty((CHUNK, MAX_CHUNKS, HEAD + 1), dtype=kv_np)
        vp[:, :, :HEAD] = (
            vc[:, h, :].reshape(MAX_CHUNKS, CHUNK, HEAD).transpose(1, 0, 2).astype(kv_np)
        )
        vp[:, :, HEAD] = 1.0
        qT = np.ascontiguousarray(
            query[:, GROUP * h : GROUP * (h + 1), :].reshape(NUM_SEQS * GROUP, HEAD).T
        )
        nkT = np.ascontiguousarray(key[:, h, :].T)  # [128, 16]
        nv = np.ascontiguousarray(value[:, h, :])  # [16, 128]
        in_maps.append(
            {"qT": qT, "ktp": ktp, "vp": vp, "nkT": nkT, "nv": nv, "ebias": eb}
        )
    return in_maps


def kernel(query, key, value, key_cache, value_cache, block_tables, seq_lens):
    from concourse.bass_utils import run_bass_kernel_spmd

    block_tables = np.asarray(block_tables)
    seq_lens_np = np.asarray(seq_lens)

    cache_key = (tuple(int(x) for x in seq_lens_np), block_tables.tobytes())
    nc = _BUILD_CACHE.get(cache_key)
    if nc is None:
        nc = _build_bass(seq_lens_np, block_tables)
        _BUILD_CACHE[cache_key] = nc

    in_maps = _prep_inputs(query, key, value, key_cache, value_cache, seq_lens_np)
    res = run_bass_kernel_spmd(nc, in_maps, core_ids=list(range(N_CORES)))

    full = np.empty((NUM_SEQS, NUM_HEADS, HEAD), dtype=np.float32)
    for h in range(N_CORES):
        o = np.asarray(res.results[h]["out"])  # [GROUP, NUM_SEQS, HEAD]
        full[:, GROUP * h : GROUP * (h + 1), :] = o.transpose(1, 0, 2)
    return full.reshape(NUM_SEQS, NUM_HEADS * HEAD)


# revision 32
# speedup vs baseline: 1.5263x; 1.0230x over previous
"""Paged GQA decode attention (vLLM-style) on 8 Trainium2 NeuronCores.

Problem (hardcoded shapes):
  query       (16, 32, 128) f32     16 seqs, 32 q heads, head 128
  key/value   (16, 8, 128)  f32     new decode token per seq, 8 kv heads
  key_cache   (4096, 16, 8, 128)    paged KV cache, block 16, 4096 blocks
  value_cache (4096, 16, 8, 128)
  block_tables(16, 256) i32         per-seq physical block list
  seq_lens    (16,) i32             context length incl. new token
  out         (16, 4096) f32        attention output, heads*head flattened

Sharding: tensor-parallel over the 8 kv heads -> core h owns kv head h and
its 4 query heads (GQA group = 4). Block tables / seq_lens replicated and
burned into the (identical-across-cores) instruction stream at build time.

Per-core algorithm (scoresT orientation, no max-subtraction -- scores are
~N(0,1) after the 1/sqrt(128) scale so exp never overflows):
  per seq s, per 128-token chunk t:
    scoresT[tok,4] = matmul(lhsT=K^T[head,tok] chunk, rhs=Q^T[head,4])
    probsT = exp(scale*scoresT + bias)      (ACT; bias column masks the tail)
    out[4,129]  += matmul(lhsT=probsT[tok,4], rhs=V[tok,129])   (PSUM accum)
  column 128 of V is a baked 1.0 -> out[:,128] is the softmax denominator.
  final: out[:, :128] * reciprocal(out[:, 128]).

Layouts prepared on the host (part of sharding):
  ktp  [128, 65536] f32  K^T: ktp[d, slot]  (slot = block*16 + offset)
  vp   [128, 512, 129]   V:  vp[p, C, d] = V[slot=128*C+p, d]; vp[p,C,128]=1
  qT   [128, 64]         qT[d, 4*s+g] = query[s, 4h+g, d]
  nkT  [128, 16]         new k transposed;  nv [16, 128] new v
  ebias[128, 32]         exp bias: col 2s = 0-vector, col 2s+1 = tail mask
TensorE consumes bf16 (FP32 matmul is 4x slower); with KV_BF16 the sharded
cache is stored bf16 in HBM (same SBUF values as the cast-during-DMA path,
half the traffic), K rides the SP HWDGE ring and V the ACT ring. The new
token's K/V is spliced into the SBUF tiles (device-side cache insert) before
the matmuls; the stale cache slot is overwritten, and positions >= L get exp
bias -30000 -> prob 0.
"""

import math

import numpy as np

NUM_SEQS = 16
NUM_HEADS = 32
NUM_KV = 8
HEAD = 128
BLOCK_SIZE = 16
NUM_BLOCKS = 4096
TOT_SLOTS = NUM_BLOCKS * BLOCK_SIZE  # 65536
GROUP = NUM_HEADS // NUM_KV  # 4
N_CORES = 8
CHUNK = 128  # tokens per matmul chunk
MAX_CHUNKS = 512  # TOT_SLOTS / CHUNK
SEQ_MAX_CHUNKS = 32  # 4096-token max context / 128

_BUILD_CACHE = {}

# Store the sharded KV cache in HBM as bf16. TensorE-facing tensors are bf16
# either way (without this flag the f32->bf16 cast happens inside the SWDGE
# DMA), so the SBUF values and the output are identical -- this only halves
# the HBM traffic.
KV_BF16 = True


def _slot_runs(block_tables, s, nchunks):
    """Physical-slot layout for tokens [0, nchunks*128) of seq s, coalesced
    into maximal runs of consecutive slots. Returns list of (dst_tok, slot0,
    length)."""
    nblk = nchunks * (CHUNK // BLOCK_SIZE)
    blocks = np.asarray(block_tables[s, :nblk], dtype=np.int64)
    slots = (blocks[:, None] * BLOCK_SIZE + np.arange(BLOCK_SIZE)[None, :]).reshape(-1)
    runs = []
    start = 0
    for i in range(1, len(slots) + 1):
        if i == len(slots) or slots[i] != slots[i - 1] + 1:
            runs.append((start, int(slots[start]), i - start))
            start = i
    return runs


def _build_bass(seq_lens, block_tables):
    import concourse.bacc as bacc
    import concourse.mybir as mybir
    import concourse.tile as tile

    f32 = mybir.dt.float32
    bf16 = mybir.dt.bfloat16
    Exp = mybir.ActivationFunctionType.Exp
    scale = 1.0 / math.sqrt(HEAD)

    seq_lens = [int(x) for x in seq_lens]
    nch = [int(math.ceil(L / CHUNK)) for L in seq_lens]

    kv_dt = bf16 if KV_BF16 else f32

    nc = bacc.Bacc()
    qT_d = nc.dram_tensor("qT", [HEAD, NUM_SEQS * GROUP], f32, kind="ExternalInput")
    ktp_d = nc.dram_tensor("ktp", [HEAD, TOT_SLOTS], kv_dt, kind="ExternalInput")
    vp_d = nc.dram_tensor("vp", [CHUNK, MAX_CHUNKS, HEAD + 1], kv_dt, kind="ExternalInput")
    nkT_d = nc.dram_tensor("nkT", [HEAD, NUM_SEQS], f32, kind="ExternalInput")
    nv_d = nc.dram_tensor("nv", [NUM_SEQS, HEAD], f32, kind="ExternalInput")
    eb_d = nc.dram_tensor("ebias", [CHUNK, 2 * NUM_SEQS], f32, kind="ExternalInput")
    out_d = nc.dram_tensor("out", [GROUP, NUM_SEQS, HEAD], f32, kind="ExternalOutput")

    with tile.TileContext(nc) as tc:
        with (
            tc.tile_pool(name="consts", bufs=1) as cpool,
            tc.tile_pool(name="kt", bufs=3) as kt_pool,
            tc.tile_pool(name="v", bufs=3) as v_pool,
            tc.tile_pool(name="probs", bufs=3) as p_pool,
            tc.tile_pool(name="fin", bufs=1) as fin_pool,
            tc.tile_pool(name="scps", bufs=3, space="PSUM") as sc_pool,
            tc.tile_pool(name="ops", bufs=3, space="PSUM") as o_pool,
        ):
            # const loads stay off gpsimd so SWDGE starts emitting the first
            # big K/V descriptors immediately; cast f32->bf16 on DVE instead
            qT_f = cpool.tile([HEAD, NUM_SEQS * GROUP], f32)
            nc.sync.dma_start(qT_f[:], qT_d[:])
            qT_sb = cpool.tile([HEAD, NUM_SEQS * GROUP], bf16)
            nc.vector.tensor_copy(qT_sb[:], qT_f[:])
            eb_sb = cpool.tile([CHUNK, 2 * NUM_SEQS], f32)
            nc.sync.dma_start(eb_sb[:], eb_d[:])
            nkT_f = cpool.tile([HEAD, NUM_SEQS], f32)
            nc.sync.dma_start(nkT_f[:], nkT_d[:])
            nkT_sb = cpool.tile([HEAD, NUM_SEQS], bf16)
            nc.vector.tensor_copy(nkT_sb[:], nkT_f[:])
            nv_f = cpool.tile([NUM_SEQS, HEAD], f32)
            nc.sync.dma_start(nv_f[:], nv_d[:])
            nv_sb = cpool.tile([NUM_SEQS, HEAD], bf16)
            nc.vector.tensor_copy(nv_sb[:], nv_f[:])
            stage = fin_pool.tile([GROUP, NUM_SEQS, HEAD + 1], f32)
            rd = fin_pool.tile([GROUP, NUM_SEQS], f32)
            osb = fin_pool.tile([GROUP, NUM_SEQS, HEAD], f32)

            # longest sequences first: the tail of the kernel is the last
            # seq's compute after its DMA lands -- make that the shortest
            order = sorted(range(NUM_SEQS), key=lambda s: -seq_lens[s])

            def issue_loads(s):
                """K/V streams + new-token splices for one seq, issued two
                seqs ahead of the consuming compute so no DMA trigger queues
                behind a compute-waiting instruction in its sequencer FIFO.
                Two parallel HWDGE rings: K on SP, V on ACT. (Both on one
                ring serializes the stream: measured 147us vs 99us; V on
                SWDGE couples with the gpsimd tiny-DMA stream: 129us.)"""
                L = seq_lens[s]
                n = nch[s]
                last = L - 1
                kt = kt_pool.tile([HEAD, SEQ_MAX_CHUNKS * CHUNK], bf16, tag="kt")
                vt = v_pool.tile([CHUNK, SEQ_MAX_CHUNKS, HEAD + 1], bf16, tag="v")
                if KV_BF16:
                    k_dma, v_dma = nc.sync.dma_start, nc.scalar.dma_start
                else:
                    k_dma = v_dma = nc.gpsimd.dma_start

                runs = _slot_runs(block_tables, s, n)
                for dst, slot0, ln in runs:
                    k_dma(kt[:, dst : dst + ln], ktp_d[:, slot0 : slot0 + ln])
                if len(runs) == 1 and runs[0][1] % CHUNK == 0:
                    c0 = runs[0][1] // CHUNK
                    v_dma(vt[:, :n, :], vp_d[:, c0 : c0 + n, :])
                else:
                    # general path: one DMA per 16-token block (block-aligned
                    # slots never straddle a 128-row physical chunk)
                    for dst, slot0, ln in runs:
                        for o in range(0, ln, BLOCK_SIZE):
                            sl = slot0 + o
                            dt_ = dst + o
                            v_dma(
                                vt[dt_ % CHUNK : dt_ % CHUNK + BLOCK_SIZE, dt_ // CHUNK, :],
                                vp_d[sl % CHUNK : sl % CHUNK + BLOCK_SIZE, sl // CHUNK, : HEAD + 1],
                            )

                # splice the new token's K/V over the stale cache slot
                # (tiny transfers stay off the HWDGE rings -- their completion
                # latency would head-of-line-block the big K/V streams)
                nc.vector.tensor_copy(kt[:, last : last + 1], nkT_sb[:, s : s + 1])
                r, c_last = last % CHUNK, last // CHUNK
                nc.gpsimd.dma_start(vt[r : r + 1, c_last, :HEAD], nv_sb[s : s + 1, :])
                return kt, vt

            PREFETCH = 2
            tiles = {}
            for si in range(min(PREFETCH, NUM_SEQS)):
                tiles[si] = issue_loads(order[si])

            for si, s in enumerate(order):
                if si + PREFETCH < NUM_SEQS:
                    tiles[si + PREFETCH] = issue_loads(order[si + PREFETCH])
                kt, vt = tiles.pop(si)
                L = seq_lens[s]
                n = nch[s]

                sc = sc_pool.tile([CHUNK, SEQ_MAX_CHUNKS * GROUP], f32, tag="sc")
                for t in range(n):
                    nc.tensor.matmul(
                        sc[:, GROUP * t : GROUP * (t + 1)],
                        kt[:, CHUNK * t : CHUNK * (t + 1)],
                        qT_sb[:, GROUP * s : GROUP * (s + 1)],
                        start=True,
                        stop=True,
                    )

                probs = p_pool.tile([CHUNK, SEQ_MAX_CHUNKS * GROUP], bf16, tag="probs")
                if n > 1:
                    nc.scalar.activation(
                        probs[:, : GROUP * (n - 1)],
                        sc[:, : GROUP * (n - 1)],
                        Exp,
                        bias=eb_sb[:, 2 * s : 2 * s + 1],
                        scale=scale,
                    )
                nc.scalar.activation(
                    probs[:, GROUP * (n - 1) : GROUP * n],
                    sc[:, GROUP * (n - 1) : GROUP * n],
                    Exp,
                    bias=eb_sb[:, 2 * s + 1 : 2 * s + 2],
                    scale=scale,
                )

                acc = o_pool.tile([GROUP, HEAD + 1], f32, tag="acc")
                for t in range(n):
                    nc.tensor.matmul(
                        acc[:],
                        probs[:, GROUP * t : GROUP * (t + 1)],
                        vt[:, t, :],
                        start=(t == 0),
                        stop=(t == n - 1),
                    )
                # per-seq finalize so the output DMA overlaps later seqs
                nc.vector.tensor_copy(stage[:, s, :], acc[:])
                nc.vector.reciprocal(rd[:, s : s + 1], stage[:, s, HEAD:])
                nc.vector.tensor_tensor(
                    osb[:, s, :],
                    stage[:, s, :HEAD],
                    rd[:, s : s + 1].to_broadcast((GROUP, HEAD)),
                    mybir.AluOpType.mult,
                )
                nc.gpsimd.dma_start(out_d[:, s, :], osb[:, s, :])

    nc.finalize()
    return nc


def _prep_inputs(query, key, value, key_cache, value_cache, seq_lens):
    """Per-core host shards. Returns list of 8 dicts of f32 arrays."""
    query = np.asarray(query, dtype=np.float32)
    key = np.asarray(key, dtype=np.float32)
    value = np.asarray(value, dtype=np.float32)
    key_cache = np.asarray(key_cache, dtype=np.float32)
    value_cache = np.asarray(value_cache, dtype=np.float32)
    seq_lens = np.asarray(seq_lens)

    # exp bias: for each seq a zero column (full chunks) and a tail-mask
    # column for the final chunk (rows >= L - 128*(nch-1) get -30000)
    eb = np.zeros((CHUNK, 2 * NUM_SEQS), dtype=np.float32)
    for s in range(NUM_SEQS):
        L = int(seq_lens[s])
        n = int(math.ceil(L / CHUNK))
        v = L - CHUNK * (n - 1)
        eb[v:, 2 * s + 1] = -30000.0

    kc = key_cache.reshape(TOT_SLOTS, NUM_KV, HEAD)
    vc = value_cache.reshape(TOT_SLOTS, NUM_KV, HEAD)
    if KV_BF16:
        import ml_dtypes

        kv_np = ml_dtypes.bfloat16
    else:
        kv_np = np.float32

    in_maps = []
    for h in range(N_CORES):
        ktp = np.ascontiguousarray(kc[:, h, :].T.astype(kv_np))  # [128, 65536]
        vp = np.empty((CHUNK, MAX_CHUNKS, HEAD + 1), dtype=kv_np)
        vp[:, :, :HEAD] = (
            vc[:, h, :].reshape(MAX_CHUNKS, CHUNK, HEAD).transpose(1, 0, 2).astype(kv_np)
        )
        vp[:, :, HEAD] = 1.0
        qT = np.ascontiguousarray(
            query[:, GROUP * h : GROUP * (h + 1), :].reshape(NUM_SEQS * GROUP, HEAD).T
        )
        nkT = np.ascontiguousarray(key[:, h, :].T)  # [128, 16]
        nv = np.ascontiguousarray(value[:, h, :])  # [16, 128]
        in_maps.append(
            {"qT": qT, "ktp": ktp, "vp": vp, "nkT": nkT, "nv": nv, "ebias": eb}
        )
    return in_maps


def kernel(query, key, value, key_cache, value_cache, block_tables, seq_lens):
    from concourse.bass_utils import run_bass_kernel_spmd

    block_tables = np.asarray(block_tables)
    seq_lens_np = np.asarray(seq_lens)

    cache_key = (tuple(int(x) for x in seq_lens_np), block_tables.tobytes())
    nc = _BUILD_CACHE.get(cache_key)
    if nc is None:
        nc = _build_bass(seq_lens_np, block_tables)
        _BUILD_CACHE[cache_key] = nc

    in_maps = _prep_inputs(query, key, value, key_cache, value_cache, seq_lens_np)
    res = run_bass_kernel_spmd(nc, in_maps, core_ids=list(range(N_CORES)))

    full = np.empty((NUM_SEQS, NUM_HEADS, HEAD), dtype=np.float32)
    for h in range(N_CORES):
        o = np.asarray(res.results[h]["out"])  # [GROUP, NUM_SEQS, HEAD]
        full[:, GROUP * h : GROUP * (h + 1), :] = o.transpose(1, 0, 2)
    return full.reshape(NUM_SEQS, NUM_HEADS * HEAD)
